# revision 1
# baseline (speedup 1.0000x reference)
"""Multi-head causal attention (B=4, S=2048, D=1024, H=16) on 8 TRN2 NeuronCores.

Sharding: core c handles batch b = c//2 and head-group hg = c%2 (8 heads each).
Each core computes Q/K/V projections for its (batch, head-group), causal
attention, and a partial output projection over its 512 head-dims.  The host
sums the two partials per batch and adds b_o.  No collectives.

Device-side layout choices:
  - x is passed transposed (xT [D, S]) so projection matmuls contract over
    partitions directly.
  - Q and K are produced transposed (QT/KT [dq, S]); scores are computed
    transposed (S^T [kpos, q]) which makes the softmax denominator a matmul
    with a ones-column (no partition reductions anywhere).
  - No max-subtraction in softmax: scaled scores are ~N(0,1), exp is safe.
  - P (=exp(scores)) and V are bf16 for the P@V matmul; everything else is
    float32r (full-rate fp32 on the PE).
"""

import sys
import os

sys.path.insert(0, "/opt/trn_rl_repo")

import numpy as np

import concourse.bacc as bacc
import concourse.mybir as mybir
import concourse.tile as tile
from concourse.bass_utils import run_bass_kernel_spmd

# The ACT table-load pass resolves each activation to the first table set
# containing it, which puts Exp (exp_and_others) and Ln
# (natural_log_exp_and_others) in different sets and reloads tables at every
# softmax normalization.  Restrict Exp/Ln to the one set that holds both so
# the whole kernel runs off a single table load.
_orig_get_tables = bacc.get_activation_tables


def _patched_tables(arch):
    t = _orig_get_tables(arch)
    for name, fns in t.items():
        if name != "natural_log_exp_and_others":
            fns.discard(mybir.ActivationFunctionType.Exp)
            fns.discard(mybir.ActivationFunctionType.Ln)
    return t


bacc.get_activation_tables = _patched_tables

B, S, D, H = 4, 2048, 1024, 16
DK = D // H          # 64
HH = H // 2          # 8 heads per core
HD = HH * DK         # 512 head-dims per core
N_CORES = 8

F32 = mybir.dt.float32
F32R = mybir.dt.float32r
BF16 = mybir.dt.bfloat16
F16 = mybir.dt.float16

SCALE = 1.0 / np.sqrt(DK)


def act_reciprocal(nc, out, in_):
    """Reciprocal on the ACT LUT (~1e-3 rel err, fine for softmax denoms).

    bass's activation() helper refuses Reciprocal for accuracy reasons;
    emit the instruction directly."""
    eng = nc.scalar
    ins = [eng.lower_ap(in_)]
    for v in (0.0, 1.0, 0.0):  # bias, scale, alpha
        ins.append(mybir.ImmediateValue(dtype=mybir.dt.float32, value=v))
    return eng.add_instruction(mybir.InstActivation(
        name=nc.get_next_instruction_name(),
        func=mybir.ActivationFunctionType.Reciprocal,
        ins=ins, outs=[eng.lower_ap(out)]))


def build_nc(s=S, interleave_pairs=True):
    """Build the per-core SPMD program.  `s` is the sequence length (tunable
    for small-scale simulation; must be a multiple of 512)."""
    assert s % 512 == 0
    n_qb = s // 512          # 512-wide q blocks
    n_t128 = s // 128        # 128-wide token tiles
    n_dt = D // 128          # din tiles (8)

    nc = bacc.Bacc("TRN2", target_bir_lowering=False, debug=False,
                   num_devices=N_CORES)

    xT = nc.dram_tensor("xT", [D, s], F16, kind="ExternalInput")
    wqT = nc.dram_tensor("wqT", [D, HD], F16, kind="ExternalInput")
    wkT = nc.dram_tensor("wkT", [D, HD], F16, kind="ExternalInput")
    wvT = nc.dram_tensor("wvT", [D, HD], F16, kind="ExternalInput")
    woT = nc.dram_tensor("woT", [HD, D], F16, kind="ExternalInput")
    out = nc.dram_tensor("out", [s, D], F32, kind="ExternalOutput")

    with tile.TileContext(nc) as tc:
        with tc.tile_pool(name="persist", bufs=1) as persist, \
             tc.tile_pool(name="wload", bufs=16) as wload, \
             tc.tile_pool(name="xtb", bufs=12) as xtb_pool, \
             tc.tile_pool(name="pT", bufs=40) as pT_pool, \
             tc.tile_pool(name="aoT", bufs=8) as aoT_pool, \
             tc.tile_pool(name="rb", bufs=4) as rb_pool, \
             tc.tile_pool(name="outsb", bufs=2) as out_pool, \
             tc.tile_pool(name="xsl", bufs=8) as xsl_pool, \
             tc.tile_pool(name="spsum", bufs=3, space="PSUM") as spsum, \
             tc.tile_pool(name="upsum", bufs=3, space="PSUM") as upsum, \
             tc.tile_pool(name="opsum", bufs=2, space="PSUM") as opsum:

            # Persistent SBUF arrays (live for the whole kernel).
            qt_sb = [persist.tile([128, s], F16, tag=f"qt{d}", name=f"qt{d}") for d in range(HD // 128)]
            # Per-head K^T tiles, zero-padded to 128 contraction rows: head h
            # occupies rows (h%2)*64..(h%2)*64+63, the other 64 rows are zero.
            # Scores matmuls can then use full 128x128 PE mode (the zero rows
            # multiply the paired head's Q rows harmlessly) -- avoiding tiling
            # mode switches, which drain the PE between instructions.
            kt_sb = [persist.tile([128, s], F16, tag=f"kt{h}", name=f"kt{h}") for h in range(HH)]
            for h in range(HH):
                z = (1 - h % 2) * 64
                nc.vector.memset(kt_sb[h][z:z + 64, :], 0.0)
            # V tiles hold [t, head, 2*dk]: cols 0-63 are V, cols 64-127 are
            # 1.0.  As the AV stationary this makes the matmul emit U^T on
            # psum rows 0-63 and the softmax denominator on rows 64-127.
            v_sb = [persist.tile([128, HH, 2 * DK], F16, tag=f"v{t}", name=f"v{t}") for t in range(n_t128)]
            wo_sb = [persist.tile([128, D], F16, tag=f"wo{d}", name=f"wo{d}") for d in range(HD // 128)]
            wv_sb = [persist.tile([128, HD], F16, tag=f"wv{i}", name=f"wv{i}") for i in range(n_dt)]

            # Weights first: they gate the first matmuls.
            w_tiles = {}
            for wdram, wkey in ((wqT, "q"), (wkT, "k")):
                for i in range(n_dt):
                    wt = wload.tile([128, HD], F16, tag="w", name="w")
                    nc.sync.dma_start(out=wt[:], in_=wdram[i * 128:(i + 1) * 128, :])
                    w_tiles[(wkey, i)] = wt
            for i in range(n_dt):
                nc.sync.dma_start(out=wv_sb[i][:], in_=wvT[i * 128:(i + 1) * 128, :])
            for d in range(HD // 128):
                nc.sync.dma_start(out=wo_sb[d][:], in_=woT[d * 128:(d + 1) * 128, :])

            def emit_proj_chains(tb, dqs):
                """Q^T and K^T projection chains for one 512-token block and
                the given dq tiles, streaming x straight from DRAM."""
                xs = []
                for i in range(n_dt):
                    t = xtb_pool.tile([128, 512], F16, tag="xtb", name="xtb")
                    nc.sync.dma_start(
                        out=t[:], in_=xT[i * 128:(i + 1) * 128,
                                         tb * 512:(tb + 1) * 512])
                    xs.append(t)
                for dq in dqs:
                    for wkey, is_k in (("q", False), ("k", True)):
                        ps = opsum.tile([128, 512], F32, tag="op", name="pp")
                        for i in range(n_dt):
                            nc.tensor.matmul(
                                ps[:],
                                lhsT=w_tiles[(wkey, i)][:, dq * 128:(dq + 1) * 128],
                                rhs=xs[i][:],
                                start=(i == 0), stop=(i == n_dt - 1),
                            )
                        if is_k:
                            for e in (0, 1):
                                nc.vector.tensor_copy(
                                    out=kt_sb[2 * dq + e][e * 64:(e + 1) * 64,
                                                          tb * 512:(tb + 1) * 512],
                                    in_=ps[e * 64:(e + 1) * 64, :])
                        else:
                            nc.vector.tensor_copy(
                                out=qt_sb[dq][:, tb * 512:(tb + 1) * 512], in_=ps[:])

            def emit_v_chain(tb):
                """V projection for one 128-token tile, spliced into the
                attention stream just before the q-block that needs it."""
                vp = opsum.tile([128, 512], F32, tag="op", name="vp")
                for i in range(n_dt):
                    xs = xsl_pool.tile([128, 128], F16, tag="xs", name="xs")
                    nc.sync.dma_start(
                        out=xs[:],
                        in_=xT[i * 128:(i + 1) * 128, tb * 128:(tb + 1) * 128])
                    nc.tensor.matmul(
                        vp[:], lhsT=xs[:], rhs=wv_sb[i][:],
                        start=(i == 0), stop=(i == n_dt - 1),
                    )
                # ones columns for the denominator, then V data (cast fp16)
                nc.vector.memset(v_sb[tb][:, :, DK:2 * DK], 1.0)
                nc.vector.tensor_copy(
                    out=v_sb[tb][:, :, 0:DK],
                    in_=vp[:].rearrange("p (h k) -> p h k", h=HH))

            def emit_score_kt(qb, hp, kt, pT):
                lo = max(kt - 4 * qb, 0) * 128
                for hh in (0, 1):
                    sp = spsum.tile([128, 512], F32, tag="sp", name="sp")
                    nc.tensor.matmul(
                        sp[:, lo:512],
                        lhsT=kt_sb[2 * hp + hh][:, kt * 128:(kt + 1) * 128],
                        rhs=qt_sb[hp][:, qb * 512 + lo:(qb + 1) * 512],
                        start=True, stop=True,
                    )
                    p = pT_pool.tile([128, 512], F16, tag="p", name="p")
                    if lo > 0:
                        # below-diagonal columns: P must be exact zeros
                        # (they are read by the full-width AV matmul)
                        nc.gpsimd.memset(p[:, 0:lo], 0.0)
                    nc.scalar.activation(
                        out=p[:, lo:512], in_=sp[:, lo:512],
                        func=mybir.ActivationFunctionType.Exp,
                        scale=float(SCALE))
                    if kt >= 4 * qb:
                        # zero strict-upper (kpos > q) region of the
                        # diagonal-crossing tile
                        nc.gpsimd.affine_select(
                            out=p[:, lo:512], in_=p[:, lo:512],
                            compare_op=mybir.AluOpType.is_ge,
                            fill=0.0, base=0, channel_multiplier=-1,
                            pattern=[[1, 512 - lo]])
                    pT[(kt, hh)] = p

            def emit_pair(cur, nxt, pT_cur, pT_next):
                """Interleave next pair's scores with current pair's AV
                chains at kt granularity: the PE gets AV matmuls to run
                while the ACT engine works through the scores' exps."""
                nkt_cur = 4 * cur[0] + 4 if cur else 0
                nkt_nxt = 4 * nxt[0] + 4 if nxt else 0
                u = {}
                ao = None
                if cur:
                    ao = aoT_pool.tile([128, 512], F16, tag="aoT", name="aoT")
                    for hh in (0, 1):
                        u[hh] = upsum.tile([128, 512], F32, tag="u", name="u")
                for kt in range(max(nkt_cur, nkt_nxt)):
                    if kt < nkt_nxt:
                        emit_score_kt(nxt[0], nxt[1], kt, pT_next)
                    if kt < nkt_cur:
                        for hh in (0, 1):
                            nc.tensor.matmul(
                                u[hh][:],
                                lhsT=v_sb[kt][:, 2 * cur[1] + hh, :],
                                rhs=pT_cur[(kt, hh)][:],
                                start=(kt == 0), stop=(kt == nkt_cur - 1),
                            )
                if cur:
                    for hh in (0, 1):
                        # rows 0-63: U^T; rows 64-127: denominator bcast.
                        # 1/l = exp(-ln(l)): ln and exp share one ACT table
                        # set, so no table reloads.
                        rb = rb_pool.tile([128, 512], F32, tag="rb", name="rb")
                        nc.scalar.activation(
                            out=rb[64:128, :], in_=u[hh][64:128, :],
                            func=mybir.ActivationFunctionType.Ln)
                        nc.scalar.activation(
                            out=rb[64:128, :], in_=rb[64:128, :],
                            func=mybir.ActivationFunctionType.Exp, scale=-1.0)
                        nc.vector.tensor_mul(
                            out=ao[hh * 64:(hh + 1) * 64, :],
                            in0=u[hh][0:64, :], in1=rb[64:128, :])
                return ao

            def emit_oproj(qb, ao_pairs):
                for qt_l in range(4):
                    qt = 4 * qb + qt_l
                    osb = out_pool.tile([128, D], F32, tag="osb", name="osb")
                    for half in range(2):
                        op = opsum.tile([128, 512], F32, tag="op", name="op")
                        for hp in range(HH // 2):
                            nc.tensor.matmul(
                                op[:],
                                lhsT=ao_pairs[hp][:, qt_l * 128:(qt_l + 1) * 128],
                                rhs=wo_sb[hp][:, half * 512:(half + 1) * 512],
                                start=(hp == 0), stop=(hp == 3),
                            )
                        nc.vector.tensor_copy(
                            out=osb[:, half * 512:(half + 1) * 512], in_=op[:])
                    nc.sync.dma_start(
                        out=out[qt * 128:(qt + 1) * 128, :], in_=osb[:])

            # Demand-driven schedule: projections for q-block tb are emitted
            # inside q-block tb-1's pairs; V chains just before the block
            # needing them; out-projections three pairs after their block.
            # dq-interleaved first block so pair (0,0) unblocks after two
            # chains.
            for dq in range(HD // 128):
                emit_proj_chains(0, [dq])
            for tb in range(4):
                emit_v_chain(tb)
            pairs = [(qb, hp) for qb in range(n_qb) for hp in range(HH // 2)]
            pT_next = {}
            emit_pair(None, pairs[0], None, pT_next)
            ao_by_qb = {qb: [] for qb in range(n_qb)}
            oproj_queue = []
            for i, (qb, hp) in enumerate(pairs):
                pT_cur, pT_next = pT_next, {}
                nxt = pairs[i + 1] if i + 1 < len(pairs) else None
                if nxt and nxt[1] == 0 and nxt[0] > 0:
                    for tb in range(4 * nxt[0], 4 * nxt[0] + 4):
                        emit_v_chain(tb)
                if oproj_queue and oproj_queue[0][1] <= i:
                    oqb, _ = oproj_queue.pop(0)
                    emit_oproj(oqb, ao_by_qb.pop(oqb))
                ao_by_qb[qb].append(emit_pair((qb, hp), nxt, pT_cur, pT_next))
                if qb + 1 < n_qb:
                    # projections for the next q-block, two dq chains per pair
                    emit_proj_chains(qb + 1, [hp])
                if hp == HH // 2 - 1:
                    oproj_queue.append((qb, i + 3))
            for oqb, _ in oproj_queue:
                emit_oproj(oqb, ao_by_qb.pop(oqb))

    nc.compile()
    return nc


_NC_CACHE = {}


def _get_nc(s=S):
    if s not in _NC_CACHE:
        _NC_CACHE[s] = build_nc(s)
    return _NC_CACHE[s]


def make_in_maps(x, w_q, w_k, w_v, w_o, s=S):
    """Host-side sharding: returns the 8 per-core input maps."""
    x = np.ascontiguousarray(np.asarray(x, dtype=np.float32))
    w_q = np.asarray(w_q, dtype=np.float32)
    w_k = np.asarray(w_k, dtype=np.float32)
    w_v = np.asarray(w_v, dtype=np.float32)
    w_o = np.asarray(w_o, dtype=np.float32)

    xTs = [np.ascontiguousarray(x[b].T.astype(np.float16)) for b in range(B)]
    wqTs = [np.ascontiguousarray(w_q[hg * HD:(hg + 1) * HD, :].T.astype(np.float16)) for hg in range(2)]
    wkTs = [np.ascontiguousarray(w_k[hg * HD:(hg + 1) * HD, :].T.astype(np.float16)) for hg in range(2)]
    wvTs = [np.ascontiguousarray(w_v[hg * HD:(hg + 1) * HD, :].T.astype(np.float16)) for hg in range(2)]
    woTs = [np.ascontiguousarray(w_o[:, hg * HD:(hg + 1) * HD].T.astype(np.float16)) for hg in range(2)]

    in_maps = []
    for c in range(N_CORES):
        b, hg = c // 2, c % 2
        in_maps.append({
            "xT": xTs[b], "wqT": wqTs[hg], "wkT": wkTs[hg],
            "wvT": wvTs[hg], "woT": woTs[hg],
        })
    return in_maps


def kernel(x, w_q, w_k, w_v, w_o, b_o):
    nc = _get_nc(S)
    in_maps = make_in_maps(x, w_q, w_k, w_v, w_o, s=S)
    res = run_bass_kernel_spmd(nc, in_maps, core_ids=list(range(N_CORES)))
    b_o = np.asarray(b_o, dtype=np.float32)
    outp = np.empty((B, S, D), dtype=np.float32)
    for b in range(B):
        outp[b] = res.results[2 * b]["out"] + res.results[2 * b + 1]["out"] + b_o
    return outp



# revision 7
# speedup vs baseline: 1.1966x; 1.1966x over previous
"""Multi-head causal attention (B=4, S=2048, D=1024, H=16) on 8 TRN2 NeuronCores.

Sharding: core c handles batch b = c//2 and head-group hg = c%2 (8 heads each).
Each core computes Q/K/V projections for its (batch, head-group), causal
attention, and a partial output projection over its 512 head-dims.  The host
sums the two partials per batch and adds b_o.  No collectives.

Device-side layout choices:
  - x is passed transposed (xT [D, S]) so projection matmuls contract over
    partitions directly.
  - Q and K are produced transposed (QT/KT [dq, S]); scores are computed
    transposed (S^T [kpos, q]) which makes the softmax denominator a matmul
    with a ones-column (no partition reductions anywhere).
  - No max-subtraction in softmax: scaled scores are ~N(0,1), exp is safe.
  - P (=exp(scores)) and V are bf16 for the P@V matmul; everything else is
    float32r (full-rate fp32 on the PE).
"""

import sys
import os

sys.path.insert(0, "/opt/trn_rl_repo")

import numpy as np

import concourse.bacc as bacc
import concourse.mybir as mybir
import concourse.tile as tile
from concourse.bass_utils import run_bass_kernel_spmd

# The ACT table-load pass resolves each activation to the first table set
# containing it, which puts Exp (exp_and_others) and Ln
# (natural_log_exp_and_others) in different sets and reloads tables at every
# softmax normalization.  Restrict Exp/Ln to the one set that holds both so
# the whole kernel runs off a single table load.
_orig_get_tables = bacc.get_activation_tables


def _patched_tables(arch):
    t = _orig_get_tables(arch)
    for name, fns in t.items():
        if name != "natural_log_exp_and_others":
            fns.discard(mybir.ActivationFunctionType.Exp)
            fns.discard(mybir.ActivationFunctionType.Ln)
    return t


bacc.get_activation_tables = _patched_tables

B, S, D, H = 4, 2048, 1024, 16
DK = D // H          # 64
HH = H // 2          # 8 heads per core
HD = HH * DK         # 512 head-dims per core
N_CORES = 8

F32 = mybir.dt.float32
F32R = mybir.dt.float32r
BF16 = mybir.dt.bfloat16
F16 = mybir.dt.float16

SCALE = 1.0 / np.sqrt(DK)


def act_reciprocal(nc, out, in_):
    """Reciprocal on the ACT LUT (~1e-3 rel err, fine for softmax denoms).

    bass's activation() helper refuses Reciprocal for accuracy reasons;
    emit the instruction directly."""
    eng = nc.scalar
    ins = [eng.lower_ap(in_)]
    for v in (0.0, 1.0, 0.0):  # bias, scale, alpha
        ins.append(mybir.ImmediateValue(dtype=mybir.dt.float32, value=v))
    return eng.add_instruction(mybir.InstActivation(
        name=nc.get_next_instruction_name(),
        func=mybir.ActivationFunctionType.Reciprocal,
        ins=ins, outs=[eng.lower_ap(out)]))


def build_nc(s=S):
    """Build the per-core SPMD program.  `s` is the sequence length (tunable
    for small-scale simulation; must be a multiple of 512)."""
    assert s % 512 == 0
    n_qb = s // 512          # 512-wide q blocks
    n_t128 = s // 128        # 128-wide token tiles
    n_dt = D // 128          # din tiles (8)

    nc = bacc.Bacc("TRN2", target_bir_lowering=False, debug=False,
                   num_devices=N_CORES)

    xT = nc.dram_tensor("xT", [D, s], F16, kind="ExternalInput")
    wqT = nc.dram_tensor("wqT", [D, HD], F16, kind="ExternalInput")
    wkT = nc.dram_tensor("wkT", [D, HD], F16, kind="ExternalInput")
    wvT = nc.dram_tensor("wvT", [D, HD], F16, kind="ExternalInput")
    woT = nc.dram_tensor("woT", [HD, D], F16, kind="ExternalInput")
    out = nc.dram_tensor("out", [s, D], F32, kind="ExternalOutput")

    with tile.TileContext(nc) as tc:
        with tc.tile_pool(name="persist", bufs=1) as persist, \
             tc.tile_pool(name="wload", bufs=16) as wload, \
             tc.tile_pool(name="xtb", bufs=16) as xtb_pool, \
             tc.tile_pool(name="pT", bufs=40) as pT_pool, \
             tc.tile_pool(name="aoT", bufs=8) as aoT_pool, \
             tc.tile_pool(name="rb", bufs=4) as rb_pool, \
             tc.tile_pool(name="outsb", bufs=2) as out_pool, \
             tc.tile_pool(name="spsum", bufs=3, space="PSUM") as spsum, \
             tc.tile_pool(name="upsum", bufs=3, space="PSUM") as upsum, \
             tc.tile_pool(name="opsum", bufs=2, space="PSUM") as opsum:

            # Persistent SBUF arrays (live for the whole kernel).
            qt_sb = [persist.tile([128, s], F16, tag=f"qt{d}", name=f"qt{d}") for d in range(HD // 128)]
            # Per-head K^T tiles, zero-padded to 128 contraction rows: head h
            # occupies rows (h%2)*64..(h%2)*64+63, the other 64 rows are zero.
            # Scores matmuls can then use full 128x128 PE mode (the zero rows
            # multiply the paired head's Q rows harmlessly) -- avoiding tiling
            # mode switches, which drain the PE between instructions.
            kt_sb = [persist.tile([128, s], F16, tag=f"kt{h}", name=f"kt{h}") for h in range(HH)]
            for h in range(HH):
                z = (1 - h % 2) * 64
                nc.vector.memset(kt_sb[h][z:z + 64, :], 0.0)
            # V tiles hold [t, head, 2*dk]: cols 0-63 are V, cols 64-127 are
            # 1.0.  As the AV stationary this makes the matmul emit U^T on
            # psum rows 0-63 and the softmax denominator on rows 64-127.
            v_sb = [persist.tile([128, HH, 2 * DK], F16, tag=f"v{t}", name=f"v{t}") for t in range(n_t128)]
            wo_sb = [persist.tile([128, D], F16, tag=f"wo{d}", name=f"wo{d}") for d in range(HD // 128)]
            wv_sb = [persist.tile([128, HD], F16, tag=f"wv{i}", name=f"wv{i}") for i in range(n_dt)]

            def load_x_tiles(tb):
                """Load the 8 din tiles of x for one 512-token block.  Shared
                by the 4 projection chains and 4 V chains of that block."""
                xs = []
                for i in range(n_dt):
                    t = xtb_pool.tile([128, 512], F16, tag="xtb", name="xtb")
                    nc.sync.dma_start(
                        out=t[:], in_=xT[i * 128:(i + 1) * 128,
                                         tb * 512:(tb + 1) * 512])
                    xs.append(t)
                return xs

            # DMA priority order: x for block 0 interleaved with w_q (both
            # gate the first Q chain), then w_k, w_v, w_o — so the PE's first
            # dependencies land first instead of behind 4 MB of weights.
            w_tiles = {}
            xs0 = []
            for i in range(n_dt):
                t = xtb_pool.tile([128, 512], F16, tag="xtb", name="xtb")
                nc.sync.dma_start(out=t[:], in_=xT[i * 128:(i + 1) * 128, 0:512])
                xs0.append(t)
                wt = wload.tile([128, HD], F16, tag="w", name="w")
                nc.sync.dma_start(out=wt[:], in_=wqT[i * 128:(i + 1) * 128, :])
                w_tiles[("q", i)] = wt
            for i in range(n_dt):
                wt = wload.tile([128, HD], F16, tag="w", name="w")
                nc.sync.dma_start(out=wt[:], in_=wkT[i * 128:(i + 1) * 128, :])
                w_tiles[("k", i)] = wt
            for i in range(n_dt):
                nc.sync.dma_start(out=wv_sb[i][:], in_=wvT[i * 128:(i + 1) * 128, :])
            for d in range(HD // 128):
                nc.sync.dma_start(out=wo_sb[d][:], in_=woT[d * 128:(d + 1) * 128, :])

            def emit_proj_chains(tb, dqs, xs):
                """Q^T and K^T projection chains for one 512-token block and
                the given dq tiles, using the block's shared x tiles."""
                for dq in dqs:
                    for wkey, is_k in (("q", False), ("k", True)):
                        ps = opsum.tile([128, 512], F32, tag="op", name="pp")
                        for i in range(n_dt):
                            nc.tensor.matmul(
                                ps[:],
                                lhsT=w_tiles[(wkey, i)][:, dq * 128:(dq + 1) * 128],
                                rhs=xs[i][:],
                                start=(i == 0), stop=(i == n_dt - 1),
                            )
                        if is_k:
                            for e in (0, 1):
                                nc.vector.tensor_copy(
                                    out=kt_sb[2 * dq + e][e * 64:(e + 1) * 64,
                                                          tb * 512:(tb + 1) * 512],
                                    in_=ps[e * 64:(e + 1) * 64, :])
                        else:
                            nc.vector.tensor_copy(
                                out=qt_sb[dq][:, tb * 512:(tb + 1) * 512], in_=ps[:])

            def emit_v_chain(tb, xs):
                """V projection for one 128-token tile, spliced into the
                attention stream just before the q-block that needs it.
                `xs` holds the 512-token-block x tiles containing this tile."""
                off = (tb % 4) * 128
                vp = opsum.tile([128, 512], F32, tag="op", name="vp")
                for i in range(n_dt):
                    nc.tensor.matmul(
                        vp[:], lhsT=xs[i][:, off:off + 128], rhs=wv_sb[i][:],
                        start=(i == 0), stop=(i == n_dt - 1),
                    )
                # ones columns for the denominator, then V data (cast fp16)
                nc.vector.memset(v_sb[tb][:, :, DK:2 * DK], 1.0)
                nc.vector.tensor_copy(
                    out=v_sb[tb][:, :, 0:DK],
                    in_=vp[:].rearrange("p (h k) -> p h k", h=HH))

            def emit_score_kt(qb, hp, kt, pT):
                lo = max(kt - 4 * qb, 0) * 128
                for hh in (0, 1):
                    sp = spsum.tile([128, 512], F32, tag="sp", name="sp")
                    nc.tensor.matmul(
                        sp[:, lo:512],
                        lhsT=kt_sb[2 * hp + hh][:, kt * 128:(kt + 1) * 128],
                        rhs=qt_sb[hp][:, qb * 512 + lo:(qb + 1) * 512],
                        start=True, stop=True,
                    )
                    p = pT_pool.tile([128, 512], F16, tag="p", name="p")
                    # cols 0:lo are never written: the AV matmul is trimmed
                    # to [lo:512], so the garbage is never read.
                    nc.scalar.activation(
                        out=p[:, lo:512], in_=sp[:, lo:512],
                        func=mybir.ActivationFunctionType.Exp,
                        scale=float(SCALE))
                    if kt >= 4 * qb:
                        # zero strict-upper (kpos > q) region of the
                        # diagonal-crossing tile
                        nc.gpsimd.affine_select(
                            out=p[:, lo:512], in_=p[:, lo:512],
                            compare_op=mybir.AluOpType.is_ge,
                            fill=0.0, base=0, channel_multiplier=-1,
                            pattern=[[1, 512 - lo]])
                    pT[(kt, hh)] = p

            def emit_pair(cur, nxt, pT_cur, pT_next):
                """Interleave next pair's scores with current pair's AV
                chains at kt granularity: the PE gets AV matmuls to run
                while the ACT engine works through the scores' exps."""
                nkt_cur = 4 * cur[0] + 4 if cur else 0
                nkt_nxt = 4 * nxt[0] + 4 if nxt else 0
                u = {}
                ao = None
                if cur:
                    ao = aoT_pool.tile([128, 512], F16, tag="aoT", name="aoT")
                    for hh in (0, 1):
                        u[hh] = upsum.tile([128, 512], F32, tag="u", name="u")
                for kt in range(max(nkt_cur, nkt_nxt)):
                    if kt < nkt_nxt:
                        emit_score_kt(nxt[0], nxt[1], kt, pT_next)
                    if kt < nkt_cur:
                        # q columns below lo are above the diagonal for this
                        # kt tile: P is zero there, so skip them.
                        lo = max(kt - 4 * cur[0], 0) * 128
                        for hh in (0, 1):
                            nc.tensor.matmul(
                                u[hh][:, lo:512],
                                lhsT=v_sb[kt][:, 2 * cur[1] + hh, :],
                                rhs=pT_cur[(kt, hh)][:, lo:512],
                                start=(kt == 0), stop=(kt == nkt_cur - 1),
                            )
                if cur:
                    for hh in (0, 1):
                        # rows 0-63: U^T; rows 64-127: denominator bcast.
                        # 1/l = exp(-ln(l)): ln and exp share one ACT table
                        # set, so no table reloads.
                        rb = rb_pool.tile([128, 512], F32, tag="rb", name="rb")
                        nc.scalar.activation(
                            out=rb[64:128, :], in_=u[hh][64:128, :],
                            func=mybir.ActivationFunctionType.Ln)
                        nc.scalar.activation(
                            out=rb[64:128, :], in_=rb[64:128, :],
                            func=mybir.ActivationFunctionType.Exp, scale=-1.0)
                        nc.vector.tensor_mul(
                            out=ao[hh * 64:(hh + 1) * 64, :],
                            in0=u[hh][0:64, :], in1=rb[64:128, :])
                return ao

            def emit_oproj(qb, ao_pairs):
                for qt_l in range(4):
                    qt = 4 * qb + qt_l
                    osb = out_pool.tile([128, D], F32, tag="osb", name="osb")
                    for half in range(2):
                        op = opsum.tile([128, 512], F32, tag="op", name="op")
                        for hp in range(HH // 2):
                            nc.tensor.matmul(
                                op[:],
                                lhsT=ao_pairs[hp][:, qt_l * 128:(qt_l + 1) * 128],
                                rhs=wo_sb[hp][:, half * 512:(half + 1) * 512],
                                start=(hp == 0), stop=(hp == 3),
                            )
                        nc.vector.tensor_copy(
                            out=osb[:, half * 512:(half + 1) * 512], in_=op[:])
                    nc.sync.dma_start(
                        out=out[qt * 128:(qt + 1) * 128, :], in_=osb[:])

            # Demand-driven schedule: projections for q-block tb are emitted
            # inside q-block tb-1's pairs; V chains just before the block
            # needing them; out-projections three pairs after their block.
            # dq-interleaved first block so pair (0,0) unblocks after two
            # chains.
            for dq in range(HD // 128):
                emit_proj_chains(0, [dq], xs0)
            for tb in range(4):
                emit_v_chain(tb, xs0)
            pairs = [(qb, hp) for qb in range(n_qb) for hp in range(HH // 2)]
            pT_next = {}
            emit_pair(None, pairs[0], None, pT_next)
            ao_by_qb = {qb: [] for qb in range(n_qb)}
            oproj_queue = []
            xs_next = None
            for i, (qb, hp) in enumerate(pairs):
                pT_cur, pT_next = pT_next, {}
                nxt = pairs[i + 1] if i + 1 < len(pairs) else None
                if nxt and nxt[1] == 0 and nxt[0] > 0:
                    for tb in range(4 * nxt[0], 4 * nxt[0] + 4):
                        emit_v_chain(tb, xs_next)
                if oproj_queue and oproj_queue[0][1] <= i:
                    oqb, _ = oproj_queue.pop(0)
                    emit_oproj(oqb, ao_by_qb.pop(oqb))
                ao_by_qb[qb].append(emit_pair((qb, hp), nxt, pT_cur, pT_next))
                if qb + 1 < n_qb:
                    # projections for the next q-block, two dq chains per pair
                    if hp == 0:
                        xs_next = load_x_tiles(qb + 1)
                    emit_proj_chains(qb + 1, [hp], xs_next)
                if hp == HH // 2 - 1:
                    oproj_queue.append((qb, i + 3))
            for oqb, _ in oproj_queue:
                emit_oproj(oqb, ao_by_qb.pop(oqb))

    nc.compile()
    return nc


_NC_CACHE = {}


def _get_nc(s=S):
    if s not in _NC_CACHE:
        _NC_CACHE[s] = build_nc(s)
    return _NC_CACHE[s]


def make_in_maps(x, w_q, w_k, w_v, w_o, s=S):
    """Host-side sharding: returns the 8 per-core input maps."""
    x = np.ascontiguousarray(np.asarray(x, dtype=np.float32))
    w_q = np.asarray(w_q, dtype=np.float32)
    w_k = np.asarray(w_k, dtype=np.float32)
    w_v = np.asarray(w_v, dtype=np.float32)
    w_o = np.asarray(w_o, dtype=np.float32)

    xTs = [np.ascontiguousarray(x[b].T.astype(np.float16)) for b in range(B)]
    wqTs = [np.ascontiguousarray(w_q[hg * HD:(hg + 1) * HD, :].T.astype(np.float16)) for hg in range(2)]
    wkTs = [np.ascontiguousarray(w_k[hg * HD:(hg + 1) * HD, :].T.astype(np.float16)) for hg in range(2)]
    wvTs = [np.ascontiguousarray(w_v[hg * HD:(hg + 1) * HD, :].T.astype(np.float16)) for hg in range(2)]
    woTs = [np.ascontiguousarray(w_o[:, hg * HD:(hg + 1) * HD].T.astype(np.float16)) for hg in range(2)]

    in_maps = []
    for c in range(N_CORES):
        b, hg = c // 2, c % 2
        in_maps.append({
            "xT": xTs[b], "wqT": wqTs[hg], "wkT": wkTs[hg],
            "wvT": wvTs[hg], "woT": woTs[hg],
        })
    return in_maps


def kernel(x, w_q, w_k, w_v, w_o, b_o):
    nc = _get_nc(S)
    in_maps = make_in_maps(x, w_q, w_k, w_v, w_o, s=S)
    res = run_bass_kernel_spmd(nc, in_maps, core_ids=list(range(N_CORES)))
    b_o = np.asarray(b_o, dtype=np.float32)
    outp = np.empty((B, S, D), dtype=np.float32)
    for b in range(B):
        outp[b] = res.results[2 * b]["out"] + res.results[2 * b + 1]["out"] + b_o
    return outp



# revision 15
# speedup vs baseline: 1.2086x; 1.0100x over previous
"""Multi-head causal attention (B=4, S=2048, D=1024, H=16) on 8 TRN2 NeuronCores.

Sharding: core c handles batch b = c//2 and head-group hg = c%2 (8 heads each).
Each core computes Q/K/V projections for its (batch, head-group), causal
attention, and a partial output projection over its 512 head-dims.  The host
sums the two partials per batch and adds b_o.  No collectives.

Device-side layout choices:
  - x is passed transposed (xT [D, S]) so projection matmuls contract over
    partitions directly.
  - Q and K are produced transposed (QT/KT [dq, S]); scores are computed
    transposed (S^T [kpos, q]) which makes the softmax denominator a matmul
    with a ones-column (no partition reductions anywhere).
  - No max-subtraction in softmax: scaled scores are ~N(0,1), exp is safe.
  - P (=exp(scores)) and V are bf16 for the P@V matmul; everything else is
    float32r (full-rate fp32 on the PE).
"""

import sys
import os

sys.path.insert(0, "/opt/trn_rl_repo")

import numpy as np

import concourse.bacc as bacc
import concourse.mybir as mybir
import concourse.tile as tile
from concourse.bass_utils import run_bass_kernel_spmd

# The ACT table-load pass resolves each activation to the first table set
# containing it, which puts Exp (exp_and_others) and Ln
# (natural_log_exp_and_others) in different sets and reloads tables at every
# softmax normalization.  Restrict Exp/Ln to the one set that holds both so
# the whole kernel runs off a single table load.
_orig_get_tables = bacc.get_activation_tables


def _patched_tables(arch):
    t = _orig_get_tables(arch)
    for name, fns in t.items():
        if name != "natural_log_exp_and_others":
            fns.discard(mybir.ActivationFunctionType.Exp)
            fns.discard(mybir.ActivationFunctionType.Ln)
    return t


bacc.get_activation_tables = _patched_tables

B, S, D, H = 4, 2048, 1024, 16
DK = D // H          # 64
HH = H // 2          # 8 heads per core
HD = HH * DK         # 512 head-dims per core
N_CORES = 8

F32 = mybir.dt.float32
F32R = mybir.dt.float32r
BF16 = mybir.dt.bfloat16
F16 = mybir.dt.float16

SCALE = 1.0 / np.sqrt(DK)


def act_reciprocal(nc, out, in_):
    """Reciprocal on the ACT LUT (~1e-3 rel err, fine for softmax denoms).

    bass's activation() helper refuses Reciprocal for accuracy reasons;
    emit the instruction directly."""
    eng = nc.scalar
    ins = [eng.lower_ap(in_)]
    for v in (0.0, 1.0, 0.0):  # bias, scale, alpha
        ins.append(mybir.ImmediateValue(dtype=mybir.dt.float32, value=v))
    return eng.add_instruction(mybir.InstActivation(
        name=nc.get_next_instruction_name(),
        func=mybir.ActivationFunctionType.Reciprocal,
        ins=ins, outs=[eng.lower_ap(out)]))


def build_nc(s=S):
    """Build the per-core SPMD program.  `s` is the sequence length (tunable
    for small-scale simulation; must be a multiple of 512)."""
    assert s % 512 == 0
    n_qb = s // 512          # 512-wide q blocks
    n_t128 = s // 128        # 128-wide token tiles
    n_dt = D // 128          # din tiles (8)

    nc = bacc.Bacc("TRN2", target_bir_lowering=False, debug=False,
                   num_devices=N_CORES)

    xT = nc.dram_tensor("xT", [D, s], F16, kind="ExternalInput")
    wqT = nc.dram_tensor("wqT", [D, HD], F16, kind="ExternalInput")
    wkT = nc.dram_tensor("wkT", [D, HD], F16, kind="ExternalInput")
    wvT = nc.dram_tensor("wvT", [D, HD], F16, kind="ExternalInput")
    woT = nc.dram_tensor("woT", [HD, D], F16, kind="ExternalInput")
    # Partial outputs in f16: the two half-head partials are summed in f32 on
    # the host, so the only cost is one f16 rounding (~5e-4) per partial.
    out = nc.dram_tensor("out", [s, D], F16, kind="ExternalOutput")

    with tile.TileContext(nc) as tc:
        with tc.tile_pool(name="persist", bufs=1) as persist, \
             tc.tile_pool(name="wload", bufs=16) as wload, \
             tc.tile_pool(name="xtb", bufs=16) as xtb_pool, \
             tc.tile_pool(name="pT", bufs=40) as pT_pool, \
             tc.tile_pool(name="aoT", bufs=8) as aoT_pool, \
             tc.tile_pool(name="rb", bufs=4) as rb_pool, \
             tc.tile_pool(name="outsb", bufs=2) as out_pool, \
             tc.tile_pool(name="spsum", bufs=3, space="PSUM") as spsum, \
             tc.tile_pool(name="upsum", bufs=3, space="PSUM") as upsum, \
             tc.tile_pool(name="opsum", bufs=2, space="PSUM") as opsum:

            # PE warm-up: the HAM clock gate keeps the PE at 1.2 GHz until it
            # has been continuously busy for a full ~3.4us activity window.
            # A burst of dummy matmuls (no DMA dependencies) warms it while
            # the first weight/x DMAs stream in; everything real then runs at
            # 2.4 GHz instead of spending the first ~30us at half rate.
            warm_sb = persist.tile([128, 512], F16, tag="warm", name="warm")
            nc.vector.memset(warm_sb[:], 0.0)
            for w in range(16):
                wp = opsum.tile([128, 512], F32, tag="op", name="warm")
                nc.tensor.matmul(wp[:], lhsT=warm_sb[:, 0:128], rhs=warm_sb[:],
                                 start=True, stop=True)

            # Persistent SBUF arrays (live for the whole kernel).
            qt_sb = [persist.tile([128, s], F16, tag=f"qt{d}", name=f"qt{d}") for d in range(HD // 128)]
            # Per-head K^T tiles, zero-padded to 128 contraction rows: head h
            # occupies rows (h%2)*64..(h%2)*64+63, the other 64 rows are zero.
            # Scores matmuls can then use full 128x128 PE mode (the zero rows
            # multiply the paired head's Q rows harmlessly) -- avoiding tiling
            # mode switches, which drain the PE between instructions.
            kt_sb = [persist.tile([128, s], F16, tag=f"kt{h}", name=f"kt{h}") for h in range(HH)]
            for h in range(HH):
                z = (1 - h % 2) * 64
                nc.vector.memset(kt_sb[h][z:z + 64, :], 0.0)
            # V tiles hold [t, head, 2*dk]: cols 0-63 are V, cols 64-127 are
            # 1.0.  As the AV stationary this makes the matmul emit U^T on
            # psum rows 0-63 and the softmax denominator on rows 64-127.
            v_sb = [persist.tile([128, HH, 2 * DK], F16, tag=f"v{t}", name=f"v{t}") for t in range(n_t128)]
            wo_sb = [persist.tile([128, D], F16, tag=f"wo{d}", name=f"wo{d}") for d in range(HD // 128)]
            wv_sb = [persist.tile([128, HD], F16, tag=f"wv{i}", name=f"wv{i}") for i in range(n_dt)]

            def load_x_tiles(tb):
                """Load the 8 din tiles of x for one 512-token block.  Shared
                by the 4 projection chains and 4 V chains of that block."""
                xs = []
                for i in range(n_dt):
                    t = xtb_pool.tile([128, 512], F16, tag="xtb", name="xtb")
                    nc.sync.dma_start(
                        out=t[:], in_=xT[i * 128:(i + 1) * 128,
                                         tb * 512:(tb + 1) * 512])
                    xs.append(t)
                return xs

            # DMA priority order: x for block 0 interleaved with w_q (both
            # gate the first Q chain), then w_k, w_v, w_o — so the PE's first
            # dependencies land first instead of behind 4 MB of weights.
            w_tiles = {}
            xs0 = []
            for i in range(n_dt):
                t = xtb_pool.tile([128, 512], F16, tag="xtb", name="xtb")
                nc.sync.dma_start(out=t[:], in_=xT[i * 128:(i + 1) * 128, 0:512])
                xs0.append(t)
                wt = wload.tile([128, HD], F16, tag="w", name="w")
                nc.sync.dma_start(out=wt[:], in_=wqT[i * 128:(i + 1) * 128, :])
                w_tiles[("q", i)] = wt
            for i in range(n_dt):
                wt = wload.tile([128, HD], F16, tag="w", name="w")
                nc.sync.dma_start(out=wt[:], in_=wkT[i * 128:(i + 1) * 128, :])
                w_tiles[("k", i)] = wt
            for i in range(n_dt):
                nc.sync.dma_start(out=wv_sb[i][:], in_=wvT[i * 128:(i + 1) * 128, :])
            for d in range(HD // 128):
                nc.sync.dma_start(out=wo_sb[d][:], in_=woT[d * 128:(d + 1) * 128, :])

            def emit_proj_chains(tb, dqs, xs):
                """Q^T and K^T projection chains for one 512-token block and
                the given dq tiles, using the block's shared x tiles."""
                for dq in dqs:
                    for wkey, is_k in (("q", False), ("k", True)):
                        ps = opsum.tile([128, 512], F32, tag="op", name="pp")
                        for i in range(n_dt):
                            nc.tensor.matmul(
                                ps[:],
                                lhsT=w_tiles[(wkey, i)][:, dq * 128:(dq + 1) * 128],
                                rhs=xs[i][:],
                                start=(i == 0), stop=(i == n_dt - 1),
                            )
                        if is_k:
                            for e in (0, 1):
                                nc.vector.tensor_copy(
                                    out=kt_sb[2 * dq + e][e * 64:(e + 1) * 64,
                                                          tb * 512:(tb + 1) * 512],
                                    in_=ps[e * 64:(e + 1) * 64, :])
                        else:
                            nc.vector.tensor_copy(
                                out=qt_sb[dq][:, tb * 512:(tb + 1) * 512], in_=ps[:])

            def emit_v_chain(tb, xs):
                """V projection for one 128-token tile, spliced into the
                attention stream just before the q-block that needs it.
                `xs` holds the 512-token-block x tiles containing this tile."""
                off = (tb % 4) * 128
                vp = opsum.tile([128, 512], F32, tag="op", name="vp")
                for i in range(n_dt):
                    nc.tensor.matmul(
                        vp[:], lhsT=xs[i][:, off:off + 128], rhs=wv_sb[i][:],
                        start=(i == 0), stop=(i == n_dt - 1),
                    )
                # ones columns for the denominator, then V data (cast fp16)
                nc.vector.memset(v_sb[tb][:, :, DK:2 * DK], 1.0)
                nc.vector.tensor_copy(
                    out=v_sb[tb][:, :, 0:DK],
                    in_=vp[:].rearrange("p (h k) -> p h k", h=HH))

            def emit_score_kt(qb, hp, kt, pT):
                lo = max(kt - 4 * qb, 0) * 128
                for hh in (0, 1):
                    sp = spsum.tile([128, 512], F32, tag="sp", name="sp")
                    nc.tensor.matmul(
                        sp[:, lo:512],
                        lhsT=kt_sb[2 * hp + hh][:, kt * 128:(kt + 1) * 128],
                        rhs=qt_sb[hp][:, qb * 512 + lo:(qb + 1) * 512],
                        start=True, stop=True,
                    )
                    p = pT_pool.tile([128, 512], F16, tag="p", name="p")
                    # cols 0:lo are never written: the AV matmul is trimmed
                    # to [lo:512], so the garbage is never read.
                    nc.scalar.activation(
                        out=p[:, lo:512], in_=sp[:, lo:512],
                        func=mybir.ActivationFunctionType.Exp,
                        scale=float(SCALE))
                    if kt >= 4 * qb:
                        # zero strict-upper (kpos > q) region of the
                        # diagonal-crossing tile
                        nc.gpsimd.affine_select(
                            out=p[:, lo:512], in_=p[:, lo:512],
                            compare_op=mybir.AluOpType.is_ge,
                            fill=0.0, base=0, channel_multiplier=-1,
                            pattern=[[1, 512 - lo]])
                    pT[(kt, hh)] = p

            def emit_pair(cur, nxt, pT_cur, pT_next):
                """Interleave next pair's scores with current pair's AV
                chains at kt granularity: the PE gets AV matmuls to run
                while the ACT engine works through the scores' exps."""
                nkt_cur = 4 * cur[0] + 4 if cur else 0
                nkt_nxt = 4 * nxt[0] + 4 if nxt else 0
                u = {}
                ao = None
                if cur:
                    ao = aoT_pool.tile([128, 512], F16, tag="aoT", name="aoT")
                    for hh in (0, 1):
                        u[hh] = upsum.tile([128, 512], F32, tag="u", name="u")
                if cur and not nxt:
                    # Last pair: no next-pair scores to interleave.  Run the
                    # AV chains head-sequential and normalize each head as
                    # soon as its chain stops, so the DVE normalize of head 0
                    # overlaps the PE chain of head 1.
                    for hh in (0, 1):
                        for kt in range(nkt_cur):
                            lo = max(kt - 4 * cur[0], 0) * 128
                            nc.tensor.matmul(
                                u[hh][:, lo:512],
                                lhsT=v_sb[kt][:, 2 * cur[1] + hh, :],
                                rhs=pT_cur[(kt, hh)][:, lo:512],
                                start=(kt == 0), stop=(kt == nkt_cur - 1),
                            )
                        rb = rb_pool.tile([128, 512], F32, tag="rb", name="rb")
                        nc.scalar.activation(
                            out=rb[64:128, :], in_=u[hh][64:128, :],
                            func=mybir.ActivationFunctionType.Ln)
                        nc.scalar.activation(
                            out=rb[64:128, :], in_=rb[64:128, :],
                            func=mybir.ActivationFunctionType.Exp, scale=-1.0)
                        nc.vector.tensor_mul(
                            out=ao[hh * 64:(hh + 1) * 64, :],
                            in0=u[hh][0:64, :], in1=rb[64:128, :])
                    return ao
                for kt in range(max(nkt_cur, nkt_nxt)):
                    if kt < nkt_nxt:
                        emit_score_kt(nxt[0], nxt[1], kt, pT_next)
                    if kt < nkt_cur:
                        # q columns below lo are above the diagonal for this
                        # kt tile: P is zero there, so skip them.
                        lo = max(kt - 4 * cur[0], 0) * 128
                        for hh in (0, 1):
                            nc.tensor.matmul(
                                u[hh][:, lo:512],
                                lhsT=v_sb[kt][:, 2 * cur[1] + hh, :],
                                rhs=pT_cur[(kt, hh)][:, lo:512],
                                start=(kt == 0), stop=(kt == nkt_cur - 1),
                            )
                if cur:
                    for hh in (0, 1):
                        # rows 0-63: U^T; rows 64-127: denominator bcast.
                        # 1/l = exp(-ln(l)): ln and exp share one ACT table
                        # set, so no table reloads.
                        rb = rb_pool.tile([128, 512], F32, tag="rb", name="rb")
                        nc.scalar.activation(
                            out=rb[64:128, :], in_=u[hh][64:128, :],
                            func=mybir.ActivationFunctionType.Ln)
                        nc.scalar.activation(
                            out=rb[64:128, :], in_=rb[64:128, :],
                            func=mybir.ActivationFunctionType.Exp, scale=-1.0)
                        nc.vector.tensor_mul(
                            out=ao[hh * 64:(hh + 1) * 64, :],
                            in0=u[hh][0:64, :], in1=rb[64:128, :])
                return ao

            def emit_oproj(qb, ao_pairs):
                for qt_l in range(4):
                    qt = 4 * qb + qt_l
                    osb = out_pool.tile([128, D], F16, tag="osb", name="osb")
                    for half in range(2):
                        op = opsum.tile([128, 512], F32, tag="op", name="op")
                        for hp in range(HH // 2):
                            nc.tensor.matmul(
                                op[:],
                                lhsT=ao_pairs[hp][:, qt_l * 128:(qt_l + 1) * 128],
                                rhs=wo_sb[hp][:, half * 512:(half + 1) * 512],
                                start=(hp == 0), stop=(hp == 3),
                            )
                        nc.vector.tensor_copy(
                            out=osb[:, half * 512:(half + 1) * 512], in_=op[:])
                    nc.sync.dma_start(
                        out=out[qt * 128:(qt + 1) * 128, :], in_=osb[:])

            # Demand-driven schedule: projections for q-block tb are emitted
            # inside q-block tb-1's pairs; V chains just before the block
            # needing them; out-projections three pairs after their block.
            # dq-interleaved first block so pair (0,0) unblocks after two
            # chains.
            for dq in range(HD // 128):
                emit_proj_chains(0, [dq], xs0)
            for tb in range(4):
                emit_v_chain(tb, xs0)
            pairs = [(qb, hp) for qb in range(n_qb) for hp in range(HH // 2)]
            pT_next = {}
            emit_pair(None, pairs[0], None, pT_next)
            ao_by_qb = {qb: [] for qb in range(n_qb)}
            oproj_queue = []
            xs_next = None
            for i, (qb, hp) in enumerate(pairs):
                pT_cur, pT_next = pT_next, {}
                nxt = pairs[i + 1] if i + 1 < len(pairs) else None
                if nxt and nxt[1] == 0 and nxt[0] > 0:
                    for tb in range(4 * nxt[0], 4 * nxt[0] + 4):
                        emit_v_chain(tb, xs_next)
                if oproj_queue and oproj_queue[0][1] <= i:
                    oqb, _ = oproj_queue.pop(0)
                    emit_oproj(oqb, ao_by_qb.pop(oqb))
                ao_by_qb[qb].append(emit_pair((qb, hp), nxt, pT_cur, pT_next))
                if qb + 1 < n_qb:
                    # projections for the next q-block, two dq chains per pair
                    if hp == 0:
                        xs_next = load_x_tiles(qb + 1)
                    emit_proj_chains(qb + 1, [hp], xs_next)
                if hp == HH // 2 - 1:
                    oproj_queue.append((qb, i + 3))
            for oqb, _ in oproj_queue:
                emit_oproj(oqb, ao_by_qb.pop(oqb))

    nc.compile()
    return nc


_NC_CACHE = {}


def _get_nc(s=S):
    if s not in _NC_CACHE:
        _NC_CACHE[s] = build_nc(s)
    return _NC_CACHE[s]


def make_in_maps(x, w_q, w_k, w_v, w_o, s=S):
    """Host-side sharding: returns the 8 per-core input maps."""
    x = np.ascontiguousarray(np.asarray(x, dtype=np.float32))
    w_q = np.asarray(w_q, dtype=np.float32)
    w_k = np.asarray(w_k, dtype=np.float32)
    w_v = np.asarray(w_v, dtype=np.float32)
    w_o = np.asarray(w_o, dtype=np.float32)

    xTs = [np.ascontiguousarray(x[b].T.astype(np.float16)) for b in range(B)]
    wqTs = [np.ascontiguousarray(w_q[hg * HD:(hg + 1) * HD, :].T.astype(np.float16)) for hg in range(2)]
    wkTs = [np.ascontiguousarray(w_k[hg * HD:(hg + 1) * HD, :].T.astype(np.float16)) for hg in range(2)]
    wvTs = [np.ascontiguousarray(w_v[hg * HD:(hg + 1) * HD, :].T.astype(np.float16)) for hg in range(2)]
    woTs = [np.ascontiguousarray(w_o[:, hg * HD:(hg + 1) * HD].T.astype(np.float16)) for hg in range(2)]

    in_maps = []
    for c in range(N_CORES):
        b, hg = c // 2, c % 2
        in_maps.append({
            "xT": xTs[b], "wqT": wqTs[hg], "wkT": wkTs[hg],
            "wvT": wvTs[hg], "woT": woTs[hg],
        })
    return in_maps


def kernel(x, w_q, w_k, w_v, w_o, b_o):
    nc = _get_nc(S)
    in_maps = make_in_maps(x, w_q, w_k, w_v, w_o, s=S)
    res = run_bass_kernel_spmd(nc, in_maps, core_ids=list(range(N_CORES)))
    b_o = np.asarray(b_o, dtype=np.float32)
    outp = np.empty((B, S, D), dtype=np.float32)
    for b in range(B):
        outp[b] = (res.results[2 * b]["out"].astype(np.float32)
                   + res.results[2 * b + 1]["out"].astype(np.float32) + b_o)
    return outp



# revision 16
# speedup vs baseline: 1.2736x; 1.0538x over previous
"""Multi-head causal attention (B=4, S=2048, D=1024, H=16) on 8 TRN2 NeuronCores.

Sharding: core c handles batch b = c//2 and head-group hg = c%2 (8 heads each).
Each core computes Q/K/V projections for its (batch, head-group), causal
attention, and a partial output projection over its 512 head-dims.  The host
sums the two partials per batch and adds b_o.  No collectives.

Device-side layout choices:
  - x is passed transposed (xT [D, S]) so projection matmuls contract over
    partitions directly; each 512-token block of x is loaded into SBUF once
    and shared by the Q/K projection chains and the V chains.
  - Q and K are produced transposed (QT/KT [dq, S]); scores are computed
    transposed (S^T [kpos, q]) which makes the softmax denominator a matmul
    with a ones-column (no partition reductions anywhere).
  - The two heads of a head-pair share one [128, 2, 512] PSUM scores tile
    (2 banks), so each kt tile needs a single exp and a single causal-mask
    select over both heads -- halving ACT/GpSimd instruction counts.
  - No max-subtraction in softmax: scaled scores are ~N(0,1), exp is safe.
  - Schedule: per kt iteration the pair loop emits next-pair scores, current
    pair AV, and one "filler" step (projection chains / V chains / output
    projection slices) pulled from a FIFO.  Fillers give the PE independent
    work at every dependency boundary and let the out-projection land in the
    exp-heavy late pairs where the PE would otherwise wait on ACT.
  - A burst of dummy matmuls at t=0 warms the PE HAM clock gate (1.2 -> 2.4
    GHz) while the first weights/x DMAs stream in.
"""

import sys
import os

sys.path.insert(0, "/opt/trn_rl_repo")

from collections import defaultdict

import numpy as np

import concourse.bacc as bacc
import concourse.mybir as mybir
import concourse.tile as tile
from concourse.bass_utils import run_bass_kernel_spmd

# The ACT table-load pass resolves each activation to the first table set
# containing it, which puts Exp (exp_and_others) and Ln
# (natural_log_exp_and_others) in different sets and reloads tables at every
# softmax normalization.  Restrict Exp/Ln to the one set that holds both so
# the whole kernel runs off a single table load.
_orig_get_tables = bacc.get_activation_tables


def _patched_tables(arch):
    t = _orig_get_tables(arch)
    for name, fns in t.items():
        if name != "natural_log_exp_and_others":
            fns.discard(mybir.ActivationFunctionType.Exp)
            fns.discard(mybir.ActivationFunctionType.Ln)
    return t


bacc.get_activation_tables = _patched_tables

B, S, D, H = 4, 2048, 1024, 16
DK = D // H          # 64
HH = H // 2          # 8 heads per core
HD = HH * DK         # 512 head-dims per core
N_CORES = 8

F32 = mybir.dt.float32
F16 = mybir.dt.float16

SCALE = 1.0 / np.sqrt(DK)


def build_nc(s=S):
    """Build the per-core SPMD program.  `s` is the sequence length (tunable
    for small-scale simulation; must be a multiple of 512)."""
    assert s % 512 == 0
    n_qb = s // 512          # 512-wide q blocks
    n_t128 = s // 128        # 128-wide token tiles
    n_dt = D // 128          # din tiles (8)

    nc = bacc.Bacc("TRN2", target_bir_lowering=False, debug=False,
                   num_devices=N_CORES)

    xT = nc.dram_tensor("xT", [D, s], F16, kind="ExternalInput")
    wqT = nc.dram_tensor("wqT", [D, HD], F16, kind="ExternalInput")
    wkT = nc.dram_tensor("wkT", [D, HD], F16, kind="ExternalInput")
    wvT = nc.dram_tensor("wvT", [D, HD], F16, kind="ExternalInput")
    woT = nc.dram_tensor("woT", [HD, D], F16, kind="ExternalInput")
    # Partial outputs in f16: the two half-head partials are summed in f32 on
    # the host, so the only cost is one f16 rounding (~5e-4) per partial.
    out = nc.dram_tensor("out", [s, D], F16, kind="ExternalOutput")

    with tile.TileContext(nc) as tc:
        with tc.tile_pool(name="persist", bufs=1) as persist, \
             tc.tile_pool(name="wload", bufs=16) as wload, \
             tc.tile_pool(name="xtb", bufs=16) as xtb_pool, \
             tc.tile_pool(name="pT", bufs=20) as pT_pool, \
             tc.tile_pool(name="aoT", bufs=12) as aoT_pool, \
             tc.tile_pool(name="rb", bufs=4) as rb_pool, \
             tc.tile_pool(name="outsb", bufs=3) as out_pool, \
             tc.tile_pool(name="spsum", bufs=2, space="PSUM") as spsum, \
             tc.tile_pool(name="upsum", bufs=3, space="PSUM") as upsum, \
             tc.tile_pool(name="opsum", bufs=1, space="PSUM") as opsum:

            # PE warm-up: the HAM clock gate keeps the PE at 1.2 GHz until it
            # has been continuously busy for a full ~3.4us activity window,
            # and the first ~12us of real work is DMA-paced (weights + x
            # arriving at ~200 GB/s effective).  A burst of dummy matmuls
            # with no DMA dependencies keeps the PE continuously busy through
            # that window so real work starts warm and gap-free.
            warm_sb = persist.tile([128, 512], F16, tag="warm", name="warm")
            nc.vector.memset(warm_sb[:], 0.0)
            for w in range(44):
                wp = upsum.tile([128, 512], F32, tag="u", name="warm")
                nc.tensor.matmul(wp[:], lhsT=warm_sb[:, 0:128], rhs=warm_sb[:],
                                 start=True, stop=True)

            # Persistent SBUF arrays (live for the whole kernel).
            qt_sb = [persist.tile([128, s], F16, tag=f"qt{d}", name=f"qt{d}") for d in range(HD // 128)]
            # Per-head K^T tiles, zero-padded to 128 contraction rows: head h
            # occupies rows (h%2)*64..(h%2)*64+63, the other 64 rows are zero.
            # Scores matmuls can then use full 128x128 PE mode (the zero rows
            # multiply the paired head's Q rows harmlessly) -- avoiding tiling
            # mode switches, which drain the PE between instructions.
            kt_sb = [persist.tile([128, s], F16, tag=f"kt{h}", name=f"kt{h}") for h in range(HH)]
            for h in range(HH):
                z = (1 - h % 2) * 64
                nc.vector.memset(kt_sb[h][z:z + 64, :], 0.0)
            # V tiles hold [t, head, 2*dk]: cols 0-63 are V, cols 64-127 are
            # 1.0.  As the AV stationary this makes the matmul emit U^T on
            # psum rows 0-63 and the softmax denominator on rows 64-127.
            v_sb = [persist.tile([128, HH, 2 * DK], F16, tag=f"v{t}", name=f"v{t}") for t in range(n_t128)]
            wo_sb = [persist.tile([128, D], F16, tag=f"wo{d}", name=f"wo{d}") for d in range(HD // 128)]
            wv_sb = [persist.tile([128, HD], F16, tag=f"wv{i}", name=f"wv{i}") for i in range(n_dt)]

            def load_x_tiles(tb):
                """Load the 8 din tiles of x for one 512-token block.  Shared
                by the 4 projection chains and 4 V chains of that block."""
                xs = []
                for i in range(n_dt):
                    t = xtb_pool.tile([128, 512], F16, tag="xtb", name="xtb")
                    nc.sync.dma_start(
                        out=t[:], in_=xT[i * 128:(i + 1) * 128,
                                         tb * 512:(tb + 1) * 512])
                    xs.append(t)
                return xs

            # DMA priority order: x for block 0 interleaved with w_q (both
            # gate the first Q chain), then w_k, w_v, w_o — so the PE's first
            # dependencies land first instead of behind 4 MB of weights.
            w_tiles = {}
            xs0 = []
            for i in range(n_dt):
                t = xtb_pool.tile([128, 512], F16, tag="xtb", name="xtb")
                nc.sync.dma_start(out=t[:], in_=xT[i * 128:(i + 1) * 128, 0:512])
                xs0.append(t)
                wt = wload.tile([128, HD], F16, tag="w", name="w")
                nc.sync.dma_start(out=wt[:], in_=wqT[i * 128:(i + 1) * 128, :])
                w_tiles[("q", i)] = wt
            for i in range(n_dt):
                wt = wload.tile([128, HD], F16, tag="w", name="w")
                nc.sync.dma_start(out=wt[:], in_=wkT[i * 128:(i + 1) * 128, :])
                w_tiles[("k", i)] = wt
            for i in range(n_dt):
                nc.sync.dma_start(out=wv_sb[i][:], in_=wvT[i * 128:(i + 1) * 128, :])
            for d in range(HD // 128):
                nc.sync.dma_start(out=wo_sb[d][:], in_=woT[d * 128:(d + 1) * 128, :])

            # ---- filler machinery -------------------------------------------
            # Independent PE work (projection chains, V chains, out-projection
            # slices) is queued as small "steps" (~4 matmuls each) and drained
            # one step per kt iteration of the pair loop.  This keeps the PE
            # busy across the scores->exp->AV dependency boundaries and places
            # out-projection work in the exp-heavy late pairs.
            filler = []

            def emit_filler_step():
                if filler:
                    filler.pop(0)()

            def finish_chain(ps, wkey, dq, tb):
                if wkey == "k":
                    for e in (0, 1):
                        nc.vector.tensor_copy(
                            out=kt_sb[2 * dq + e][e * 64:(e + 1) * 64,
                                                  tb * 512:(tb + 1) * 512],
                            in_=ps[e * 64:(e + 1) * 64, :])
                else:
                    nc.vector.tensor_copy(
                        out=qt_sb[dq][:, tb * 512:(tb + 1) * 512], in_=ps[:])

            def chain_mms(ps, wkey, dq, xs, i0, i1):
                for i in range(i0, i1):
                    nc.tensor.matmul(
                        ps[:],
                        lhsT=w_tiles[(wkey, i)][:, dq * 128:(dq + 1) * 128],
                        rhs=xs[i][:],
                        start=(i == 0), stop=(i == n_dt - 1),
                    )

            def emit_chain_bulk(tb, dq, xs, pool):
                """Q then K projection chain for (block tb, dq), emitted
                back-to-back (used in the prelude, psum from `pool`)."""
                for wkey in ("q", "k"):
                    ps = pool.tile([128, 512], F32, tag="u", name="pp")
                    chain_mms(ps, wkey, dq, xs, 0, n_dt)
                    finish_chain(ps, wkey, dq, tb)

            def chain_steps(tb, dq, xs):
                """The same Q+K chains as 4 filler steps of 4 matmuls each."""
                steps = []
                for wkey in ("q", "k"):
                    st = {}
                    def s1(wkey=wkey, st=st):
                        st["ps"] = opsum.tile([128, 512], F32, tag="op", name="pp")
                        chain_mms(st["ps"], wkey, dq, xs, 0, 4)
                    def s2(wkey=wkey, st=st):
                        chain_mms(st["ps"], wkey, dq, xs, 4, n_dt)
                        finish_chain(st["ps"], wkey, dq, tb)
                    steps += [s1, s2]
                return steps

            def v_mms(vp, tb, xs, i0, i1):
                off = (tb % 4) * 128
                for i in range(i0, i1):
                    nc.tensor.matmul(
                        vp[:], lhsT=xs[i][:, off:off + 128], rhs=wv_sb[i][:],
                        start=(i == 0), stop=(i == n_dt - 1),
                    )

            def finish_v(vp, tb):
                # ones columns for the denominator, then V data (cast fp16)
                nc.vector.memset(v_sb[tb][:, :, DK:2 * DK], 1.0)
                nc.vector.tensor_copy(
                    out=v_sb[tb][:, :, 0:DK],
                    in_=vp[:].rearrange("p (h k) -> p h k", h=HH))

            def emit_v_bulk(tb, xs, pool):
                vp = pool.tile([128, 512], F32, tag="u", name="vp")
                v_mms(vp, tb, xs, 0, n_dt)
                finish_v(vp, tb)

            def v_steps(tb, xs):
                st = {}
                def s1():
                    st["vp"] = opsum.tile([128, 512], F32, tag="op", name="vp")
                    v_mms(st["vp"], tb, xs, 0, 4)
                def s2():
                    v_mms(st["vp"], tb, xs, 4, n_dt)
                    finish_v(st["vp"], tb)
                return [s1, s2]

            def oproj_unit(qb, ao_pairs, qt_l, pool):
                """One 128-token slice of the out-projection: 2 steps of 4
                matmuls (one per 512-wide output half)."""
                qt = 4 * qb + qt_l
                st = {}
                def half(h):
                    def f():
                        if h == 0:
                            st["osb"] = out_pool.tile([128, D], F16,
                                                      tag="osb", name="osb")
                        op = pool.tile(
                            [128, 512], F32,
                            tag="u" if pool is upsum else "op", name="op")
                        for hp in range(HH // 2):
                            nc.tensor.matmul(
                                op[:],
                                lhsT=ao_pairs[hp][:, qt_l * 128:(qt_l + 1) * 128],
                                rhs=wo_sb[hp][:, h * 512:(h + 1) * 512],
                                start=(hp == 0), stop=(hp == HH // 2 - 1),
                            )
                        nc.vector.tensor_copy(
                            out=st["osb"][:, h * 512:(h + 1) * 512], in_=op[:])
                        if h == 1:
                            nc.sync.dma_start(
                                out=out[qt * 128:(qt + 1) * 128, :],
                                in_=st["osb"][:])
                    return f
                return [half(0), half(1)]

            # ---- attention ---------------------------------------------------

            def emit_score_kt(qb, hp, kt, pT):
                """Scores for both heads of pair hp against one 128-wide kt
                tile: two matmuls into one [128, 2, 512] psum tile (2 banks),
                then a single exp and a single causal select over both."""
                lo = max(kt - 4 * qb, 0) * 128
                sp = spsum.tile([128, 2, 512], F32, tag="sp", name="sp")
                for hh in (0, 1):
                    nc.tensor.matmul(
                        sp[:, hh, lo:512],
                        lhsT=kt_sb[2 * hp + hh][:, kt * 128:(kt + 1) * 128],
                        rhs=qt_sb[hp][:, qb * 512 + lo:(qb + 1) * 512],
                        start=True, stop=True,
                    )
                p = pT_pool.tile([128, 2, 512], F16, tag="p", name="p")
                # cols 0:lo are never written: the AV matmul is trimmed to
                # [lo:512], so the garbage is never read.
                nc.scalar.activation(
                    out=p[:, :, lo:512], in_=sp[:, :, lo:512],
                    func=mybir.ActivationFunctionType.Exp,
                    scale=float(SCALE))
                if kt >= 4 * qb:
                    # zero strict-upper (kpos > q) region of the
                    # diagonal-crossing tile; same mask for both heads
                    # (coefficient 0 on the head dim).
                    nc.gpsimd.affine_select(
                        out=p[:, :, lo:512], in_=p[:, :, lo:512],
                        compare_op=mybir.AluOpType.is_ge,
                        fill=0.0, base=0, channel_multiplier=-1,
                        pattern=[[0, 2], [1, 512 - lo]])
                pT[kt] = p

            def av_mm(u, cur, kt, hh, nkt_cur, pT_cur):
                # q columns below lo are above the diagonal for this kt
                # tile: P is zero there, so skip them.
                lo = max(kt - 4 * cur[0], 0) * 128
                nc.tensor.matmul(
                    u[:, lo:512],
                    lhsT=v_sb[kt][:, 2 * cur[1] + hh, :],
                    rhs=pT_cur[kt][:, hh, lo:512],
                    start=(kt == 0), stop=(kt == nkt_cur - 1),
                )

            def finalize_head(u, ao, hh):
                # rows 0-63: U^T; rows 64-127: denominator bcast.
                # 1/l = exp(-ln(l)): ln and exp share one ACT table set,
                # so no table reloads.
                rb = rb_pool.tile([128, 512], F32, tag="rb", name="rb")
                nc.scalar.activation(
                    out=rb[64:128, :], in_=u[64:128, :],
                    func=mybir.ActivationFunctionType.Ln)
                nc.scalar.activation(
                    out=rb[64:128, :], in_=rb[64:128, :],
                    func=mybir.ActivationFunctionType.Exp, scale=-1.0)
                nc.vector.tensor_mul(
                    out=ao[hh * 64:(hh + 1) * 64, :],
                    in0=u[0:64, :], in1=rb[64:128, :])

            def emit_pair(cur, nxt, pT_cur, pT_next):
                """Interleave next pair's scores with current pair's AV
                chains at kt granularity, plus one filler step per
                iteration."""
                nkt_cur = 4 * cur[0] + 4 if cur else 0
                nkt_nxt = 4 * nxt[0] + 4 if nxt else 0
                u = {}
                ao = None
                if cur:
                    ao = aoT_pool.tile([128, 512], F16, tag="aoT", name="aoT")
                    for hh in (0, 1):
                        u[hh] = upsum.tile([128, 512], F32, tag="u", name="u")
                if cur and not nxt:
                    # Last pair: no next-pair scores to interleave.  Run the
                    # AV chains head-sequential and normalize each head as
                    # soon as its chain stops, so the normalize of head 0
                    # overlaps the PE chain of head 1.
                    for hh in (0, 1):
                        for kt in range(nkt_cur):
                            av_mm(u[hh], cur, kt, hh, nkt_cur, pT_cur)
                            emit_filler_step()
                        finalize_head(u[hh], ao, hh)
                    return ao
                for kt in range(max(nkt_cur, nkt_nxt)):
                    if kt < nkt_nxt:
                        emit_score_kt(nxt[0], nxt[1], kt, pT_next)
                    if kt < nkt_cur:
                        for hh in (0, 1):
                            av_mm(u[hh], cur, kt, hh, nkt_cur, pT_cur)
                    emit_filler_step()
                if cur:
                    for hh in (0, 1):
                        finalize_head(u[hh], ao, hh)
                return ao

            # ---- schedule ----------------------------------------------------
            # Prelude: block-0 projections and V chains in bulk (psum through
            # upsum so back-to-back chains double-buffer), dq-interleaved so
            # pair (0,0) unblocks after the first chains.
            for dq in range(HD // 128):
                emit_chain_bulk(0, dq, xs0, upsum)
            for tb in range(4):
                emit_v_bulk(tb, xs0, upsum)

            pairs = [(qb, hp) for qb in range(n_qb) for hp in range(HH // 2)]
            # out-projection slices are placed into the exp-heavy late pairs
            oproj_place = defaultdict(list)
            for qb in range(n_qb - 1):
                for qt_l in range(4):
                    idx = min(4 * qb + 6 + qt_l, len(pairs) - 2)
                    oproj_place[idx].append((qb, qt_l))

            pT_next = {}
            emit_pair(None, pairs[0], None, pT_next)
            ao_by_qb = {qb: [] for qb in range(n_qb)}
            xs_next = None
            for i, (qb, hp) in enumerate(pairs):
                pT_cur, pT_next = pT_next, {}
                nxt = pairs[i + 1] if i + 1 < len(pairs) else None
                if qb + 1 < n_qb and hp == 0:
                    xs_next = load_x_tiles(qb + 1)
                # queue filler: projection chains for the next block (one dq
                # per pair), V chains late in the block, out-projections per
                # the placement table.
                for oqb, qt_l in oproj_place.get(i, ()):
                    filler.extend(oproj_unit(oqb, ao_by_qb[oqb], qt_l, opsum))
                if qb + 1 < n_qb:
                    filler.extend(chain_steps(qb + 1, hp, xs_next))
                    if hp == HH // 2 - 1:
                        for tb in range(4 * (qb + 1), 4 * (qb + 1) + 4):
                            filler.extend(v_steps(tb, xs_next))
                ao_by_qb[qb].append(emit_pair((qb, hp), nxt, pT_cur, pT_next))
            while filler:
                emit_filler_step()
            # tail: out-projection of the last block (psum through upsum so
            # the four slices pipeline)
            for qt_l in range(4):
                for step in oproj_unit(n_qb - 1, ao_by_qb[n_qb - 1], qt_l, upsum):
                    step()

    nc.compile()
    return nc


_NC_CACHE = {}


def _get_nc(s=S):
    if s not in _NC_CACHE:
        _NC_CACHE[s] = build_nc(s)
    return _NC_CACHE[s]


def make_in_maps(x, w_q, w_k, w_v, w_o, s=S):
    """Host-side sharding: returns the 8 per-core input maps."""
    x = np.ascontiguousarray(np.asarray(x, dtype=np.float32))
    w_q = np.asarray(w_q, dtype=np.float32)
    w_k = np.asarray(w_k, dtype=np.float32)
    w_v = np.asarray(w_v, dtype=np.float32)
    w_o = np.asarray(w_o, dtype=np.float32)

    xTs = [np.ascontiguousarray(x[b].T.astype(np.float16)) for b in range(B)]
    wqTs = [np.ascontiguousarray(w_q[hg * HD:(hg + 1) * HD, :].T.astype(np.float16)) for hg in range(2)]
    wkTs = [np.ascontiguousarray(w_k[hg * HD:(hg + 1) * HD, :].T.astype(np.float16)) for hg in range(2)]
    wvTs = [np.ascontiguousarray(w_v[hg * HD:(hg + 1) * HD, :].T.astype(np.float16)) for hg in range(2)]
    woTs = [np.ascontiguousarray(w_o[:, hg * HD:(hg + 1) * HD].T.astype(np.float16)) for hg in range(2)]

    in_maps = []
    for c in range(N_CORES):
        b, hg = c // 2, c % 2
        in_maps.append({
            "xT": xTs[b], "wqT": wqTs[hg], "wkT": wkTs[hg],
            "wvT": wvTs[hg], "woT": woTs[hg],
        })
    return in_maps


def kernel(x, w_q, w_k, w_v, w_o, b_o):
    nc = _get_nc(S)
    in_maps = make_in_maps(x, w_q, w_k, w_v, w_o, s=S)
    res = run_bass_kernel_spmd(nc, in_maps, core_ids=list(range(N_CORES)))
    b_o = np.asarray(b_o, dtype=np.float32)
    outp = np.empty((B, S, D), dtype=np.float32)
    for b in range(B):
        outp[b] = (res.results[2 * b]["out"].astype(np.float32)
                   + res.results[2 * b + 1]["out"].astype(np.float32) + b_o)
    return outp


# revision 20
# speedup vs baseline: 1.2810x; 1.0058x over previous
"""Multi-head causal attention (B=4, S=2048, D=1024, H=16) on 8 TRN2 NeuronCores.

Sharding: core c handles batch b = c//2 and head-group hg = c%2 (8 heads each).
Each core computes Q/K/V projections for its (batch, head-group), causal
attention, and a partial output projection over its 512 head-dims.  The host
sums the two partials per batch and adds b_o.  No collectives.

Device-side layout choices:
  - x is passed transposed (xT [D, S]) so projection matmuls contract over
    partitions directly; each 512-token block of x is loaded into SBUF once
    and shared by the Q/K projection chains and the V chains.
  - Q and K are produced transposed (QT/KT [dq, S]); scores are computed
    transposed (S^T [kpos, q]) which makes the softmax denominator a matmul
    with a ones-column (no partition reductions anywhere).
  - The two heads of a head-pair share one [128, 2, 512] PSUM scores tile
    (2 banks), so each kt tile needs a single exp and a single causal-mask
    select over both heads -- halving ACT/GpSimd instruction counts.
  - No max-subtraction in softmax: scaled scores are ~N(0,1), exp is safe.
  - Schedule: per kt iteration the pair loop emits next-pair scores, current
    pair AV, and one "filler" step (projection chains / V chains / output
    projection slices) pulled from a FIFO.  Fillers give the PE independent
    work at every dependency boundary and let the out-projection land in the
    exp-heavy late pairs where the PE would otherwise wait on ACT.
  - A burst of dummy matmuls at t=0 warms the PE HAM clock gate (1.2 -> 2.4
    GHz) while the first weights/x DMAs stream in.
"""

import sys
import os

sys.path.insert(0, "/opt/trn_rl_repo")

from collections import defaultdict

import numpy as np

import concourse.bacc as bacc
import concourse.mybir as mybir
import concourse.tile as tile
from concourse.bass_utils import run_bass_kernel_spmd

# The ACT table-load pass resolves each activation to the first table set
# containing it, which puts Exp (exp_and_others) and Ln
# (natural_log_exp_and_others) in different sets and reloads tables at every
# softmax normalization.  Restrict Exp/Ln to the one set that holds both so
# the whole kernel runs off a single table load.
_orig_get_tables = bacc.get_activation_tables


def _patched_tables(arch):
    t = _orig_get_tables(arch)
    for name, fns in t.items():
        if name != "natural_log_exp_and_others":
            fns.discard(mybir.ActivationFunctionType.Exp)
            fns.discard(mybir.ActivationFunctionType.Ln)
    return t


bacc.get_activation_tables = _patched_tables

B, S, D, H = 4, 2048, 1024, 16
DK = D // H          # 64
HH = H // 2          # 8 heads per core
HD = HH * DK         # 512 head-dims per core
N_CORES = 8

F32 = mybir.dt.float32
F16 = mybir.dt.float16

SCALE = 1.0 / np.sqrt(DK)


def build_nc(s=S):
    """Build the per-core SPMD program.  `s` is the sequence length (tunable
    for small-scale simulation; must be a multiple of 512)."""
    assert s % 512 == 0
    n_qb = s // 512          # 512-wide q blocks
    n_t128 = s // 128        # 128-wide token tiles
    n_dt = D // 128          # din tiles (8)

    nc = bacc.Bacc("TRN2", target_bir_lowering=False, debug=False,
                   num_devices=N_CORES)

    xT = nc.dram_tensor("xT", [D, s], F16, kind="ExternalInput")
    wqT = nc.dram_tensor("wqT", [D, HD], F16, kind="ExternalInput")
    wkT = nc.dram_tensor("wkT", [D, HD], F16, kind="ExternalInput")
    wvT = nc.dram_tensor("wvT", [D, HD], F16, kind="ExternalInput")
    woT = nc.dram_tensor("woT", [HD, D], F16, kind="ExternalInput")
    # Partial outputs in f16: the two half-head partials are summed in f32 on
    # the host, so the only cost is one f16 rounding (~5e-4) per partial.
    out = nc.dram_tensor("out", [s, D], F16, kind="ExternalOutput")

    with tile.TileContext(nc) as tc:
        with tc.tile_pool(name="persist", bufs=1) as persist, \
             tc.tile_pool(name="wload", bufs=16) as wload, \
             tc.tile_pool(name="xtb", bufs=16) as xtb_pool, \
             tc.tile_pool(name="pT", bufs=20) as pT_pool, \
             tc.tile_pool(name="aoT", bufs=12) as aoT_pool, \
             tc.tile_pool(name="rb", bufs=4) as rb_pool, \
             tc.tile_pool(name="outsb", bufs=3) as out_pool, \
             tc.tile_pool(name="spsum", bufs=2, space="PSUM") as spsum, \
             tc.tile_pool(name="upsum", bufs=3, space="PSUM") as upsum, \
             tc.tile_pool(name="opsum", bufs=1, space="PSUM") as opsum:

            # PE warm-up: the HAM clock gate keeps the PE at 1.2 GHz until it
            # has been continuously busy for a full ~3.4us activity window,
            # and the first ~12us of real work is DMA-paced (weights + x
            # arriving at ~200 GB/s effective).  A burst of dummy matmuls
            # with no DMA dependencies keeps the PE continuously busy through
            # that window so real work starts warm and gap-free.
            warm_sb = persist.tile([128, 512], F16, tag="warm", name="warm")
            nc.vector.memset(warm_sb[:], 0.0)
            for w in range(32):
                wp = upsum.tile([128, 512], F32, tag="u", name="warm")
                nc.tensor.matmul(wp[:], lhsT=warm_sb[:, 0:128], rhs=warm_sb[:],
                                 start=True, stop=True)

            # Persistent SBUF arrays (live for the whole kernel).
            qt_sb = [persist.tile([128, s], F16, tag=f"qt{d}", name=f"qt{d}") for d in range(HD // 128)]
            # Per-head K^T tiles, zero-padded to 128 contraction rows: head h
            # occupies rows (h%2)*64..(h%2)*64+63, the other 64 rows are zero.
            # Scores matmuls can then use full 128x128 PE mode (the zero rows
            # multiply the paired head's Q rows harmlessly) -- avoiding tiling
            # mode switches, which drain the PE between instructions.
            kt_sb = [persist.tile([128, s], F16, tag=f"kt{h}", name=f"kt{h}") for h in range(HH)]
            for h in range(HH):
                z = (1 - h % 2) * 64
                nc.vector.memset(kt_sb[h][z:z + 64, :], 0.0)
            # V tiles hold [t, head, 2*dk]: cols 0-63 are V, cols 64-127 are
            # 1.0.  As the AV stationary this makes the matmul emit U^T on
            # psum rows 0-63 and the softmax denominator on rows 64-127.
            v_sb = [persist.tile([128, HH, 2 * DK], F16, tag=f"v{t}", name=f"v{t}") for t in range(n_t128)]
            wo_sb = [persist.tile([128, D], F16, tag=f"wo{d}", name=f"wo{d}") for d in range(HD // 128)]
            wv_sb = [persist.tile([128, HD], F16, tag=f"wv{i}", name=f"wv{i}") for i in range(n_dt)]

            def load_x_tiles(tb):
                """Load the 8 din tiles of x for one 512-token block.  Shared
                by the 4 projection chains and 4 V chains of that block."""
                xs = []
                for i in range(n_dt):
                    t = xtb_pool.tile([128, 512], F16, tag="xtb", name="xtb")
                    nc.sync.dma_start(
                        out=t[:], in_=xT[i * 128:(i + 1) * 128,
                                         tb * 512:(tb + 1) * 512])
                    xs.append(t)
                return xs

            # DMA priority order: x for block 0 interleaved with w_q (both
            # gate the first Q chain), then w_k, w_v, w_o — so the PE's first
            # dependencies land first instead of behind 4 MB of weights.
            w_tiles = {}
            xs0 = []
            for i in range(n_dt):
                t = xtb_pool.tile([128, 512], F16, tag="xtb", name="xtb")
                nc.sync.dma_start(out=t[:], in_=xT[i * 128:(i + 1) * 128, 0:512])
                xs0.append(t)
                wt = wload.tile([128, HD], F16, tag="w", name="w")
                nc.sync.dma_start(out=wt[:], in_=wqT[i * 128:(i + 1) * 128, :])
                w_tiles[("q", i)] = wt
            for i in range(n_dt):
                wt = wload.tile([128, HD], F16, tag="w", name="w")
                nc.sync.dma_start(out=wt[:], in_=wkT[i * 128:(i + 1) * 128, :])
                w_tiles[("k", i)] = wt
            for i in range(n_dt):
                nc.sync.dma_start(out=wv_sb[i][:], in_=wvT[i * 128:(i + 1) * 128, :])
            for d in range(HD // 128):
                nc.sync.dma_start(out=wo_sb[d][:], in_=woT[d * 128:(d + 1) * 128, :])

            # ---- filler machinery -------------------------------------------
            # Independent PE work (projection chains, V chains, out-projection
            # slices) is queued as small "steps" (~4 matmuls each) and drained
            # one step per kt iteration of the pair loop.  This keeps the PE
            # busy across the scores->exp->AV dependency boundaries and places
            # out-projection work in the exp-heavy late pairs.
            filler = []

            def emit_filler_step():
                if filler:
                    filler.pop(0)()

            def finish_chain(ps, wkey, dq, tb):
                if wkey == "k":
                    for e in (0, 1):
                        nc.vector.tensor_copy(
                            out=kt_sb[2 * dq + e][e * 64:(e + 1) * 64,
                                                  tb * 512:(tb + 1) * 512],
                            in_=ps[e * 64:(e + 1) * 64, :])
                else:
                    nc.vector.tensor_copy(
                        out=qt_sb[dq][:, tb * 512:(tb + 1) * 512], in_=ps[:])

            def chain_mms(ps, wkey, dq, xs, i0, i1):
                for i in range(i0, i1):
                    nc.tensor.matmul(
                        ps[:],
                        lhsT=w_tiles[(wkey, i)][:, dq * 128:(dq + 1) * 128],
                        rhs=xs[i][:],
                        start=(i == 0), stop=(i == n_dt - 1),
                    )

            def emit_chain_bulk(tb, dq, xs, pool):
                """Q then K projection chain for (block tb, dq), emitted
                back-to-back (used in the prelude, psum from `pool`)."""
                for wkey in ("q", "k"):
                    ps = pool.tile([128, 512], F32, tag="u", name="pp")
                    chain_mms(ps, wkey, dq, xs, 0, n_dt)
                    finish_chain(ps, wkey, dq, tb)

            def chain_steps(tb, dq, xs):
                """The same Q+K chains as 4 filler steps of 4 matmuls each."""
                steps = []
                for wkey in ("q", "k"):
                    st = {}
                    def s1(wkey=wkey, st=st):
                        st["ps"] = opsum.tile([128, 512], F32, tag="op", name="pp")
                        chain_mms(st["ps"], wkey, dq, xs, 0, 4)
                    def s2(wkey=wkey, st=st):
                        chain_mms(st["ps"], wkey, dq, xs, 4, n_dt)
                        finish_chain(st["ps"], wkey, dq, tb)
                    steps += [s1, s2]
                return steps

            def v_mms(vp, tb, xs, i0, i1):
                off = (tb % 4) * 128
                for i in range(i0, i1):
                    nc.tensor.matmul(
                        vp[:], lhsT=xs[i][:, off:off + 128], rhs=wv_sb[i][:],
                        start=(i == 0), stop=(i == n_dt - 1),
                    )

            def finish_v(vp, tb):
                # ones columns for the denominator, then V data (cast fp16)
                nc.vector.memset(v_sb[tb][:, :, DK:2 * DK], 1.0)
                nc.vector.tensor_copy(
                    out=v_sb[tb][:, :, 0:DK],
                    in_=vp[:].rearrange("p (h k) -> p h k", h=HH))

            def emit_v_bulk(tb, xs, pool):
                vp = pool.tile([128, 512], F32, tag="u", name="vp")
                v_mms(vp, tb, xs, 0, n_dt)
                finish_v(vp, tb)

            def v_steps(tb, xs):
                st = {}
                def s1():
                    st["vp"] = opsum.tile([128, 512], F32, tag="op", name="vp")
                    v_mms(st["vp"], tb, xs, 0, 4)
                def s2():
                    v_mms(st["vp"], tb, xs, 4, n_dt)
                    finish_v(st["vp"], tb)
                return [s1, s2]

            def oproj_unit(qb, ao_pairs, qt_l, pool):
                """One 128-token slice of the out-projection: 2 steps of 4
                matmuls (one per 512-wide output half)."""
                qt = 4 * qb + qt_l
                st = {}
                def half(h):
                    def f():
                        if h == 0:
                            st["osb"] = out_pool.tile([128, D], F16,
                                                      tag="osb", name="osb")
                        op = pool.tile(
                            [128, 512], F32,
                            tag="u" if pool is upsum else "op", name="op")
                        for hp in range(HH // 2):
                            nc.tensor.matmul(
                                op[:],
                                lhsT=ao_pairs[hp][:, qt_l * 128:(qt_l + 1) * 128],
                                rhs=wo_sb[hp][:, h * 512:(h + 1) * 512],
                                start=(hp == 0), stop=(hp == HH // 2 - 1),
                            )
                        nc.vector.tensor_copy(
                            out=st["osb"][:, h * 512:(h + 1) * 512], in_=op[:])
                        # per-half DMA: the last output transfer starts a
                        # copy earlier, shortening the drain tail
                        nc.sync.dma_start(
                            out=out[qt * 128:(qt + 1) * 128,
                                    h * 512:(h + 1) * 512],
                            in_=st["osb"][:, h * 512:(h + 1) * 512])
                    return f
                return [half(0), half(1)]

            # ---- attention ---------------------------------------------------

            def emit_score_kt(qb, hp, kt, pT):
                """Scores for both heads of pair hp against one 128-wide kt
                tile: two matmuls into one [128, 2, 512] psum tile (2 banks),
                then a single exp and a single causal select over both."""
                lo = max(kt - 4 * qb, 0) * 128
                sp = spsum.tile([128, 2, 512], F32, tag="sp", name="sp")
                for hh in (0, 1):
                    nc.tensor.matmul(
                        sp[:, hh, lo:512],
                        lhsT=kt_sb[2 * hp + hh][:, kt * 128:(kt + 1) * 128],
                        rhs=qt_sb[hp][:, qb * 512 + lo:(qb + 1) * 512],
                        start=True, stop=True,
                    )
                p = pT_pool.tile([128, 2, 512], F16, tag="p", name="p")
                # cols 0:lo are never written: the AV matmul is trimmed to
                # [lo:512], so the garbage is never read.
                nc.scalar.activation(
                    out=p[:, :, lo:512], in_=sp[:, :, lo:512],
                    func=mybir.ActivationFunctionType.Exp,
                    scale=float(SCALE))
                if kt >= 4 * qb:
                    # zero strict-upper (kpos > q) region of the
                    # diagonal-crossing tile; same mask for both heads
                    # (coefficient 0 on the head dim).
                    nc.gpsimd.affine_select(
                        out=p[:, :, lo:512], in_=p[:, :, lo:512],
                        compare_op=mybir.AluOpType.is_ge,
                        fill=0.0, base=0, channel_multiplier=-1,
                        pattern=[[0, 2], [1, 512 - lo]])
                pT[kt] = p

            def av_mm(u, cur, kt, hh, nkt_cur, pT_cur):
                # q columns below lo are above the diagonal for this kt
                # tile: P is zero there, so skip them.
                lo = max(kt - 4 * cur[0], 0) * 128
                nc.tensor.matmul(
                    u[:, lo:512],
                    lhsT=v_sb[kt][:, 2 * cur[1] + hh, :],
                    rhs=pT_cur[kt][:, hh, lo:512],
                    start=(kt == 0), stop=(kt == nkt_cur - 1),
                )

            def finalize_head(u, ao, hh):
                # rows 0-63: U^T; rows 64-127: denominator bcast.
                # 1/l = exp(-ln(l)): ln and exp share one ACT table set,
                # so no table reloads.
                rb = rb_pool.tile([128, 512], F32, tag="rb", name="rb")
                nc.scalar.activation(
                    out=rb[64:128, :], in_=u[64:128, :],
                    func=mybir.ActivationFunctionType.Ln)
                nc.scalar.activation(
                    out=rb[64:128, :], in_=rb[64:128, :],
                    func=mybir.ActivationFunctionType.Exp, scale=-1.0)
                nc.vector.tensor_mul(
                    out=ao[hh * 64:(hh + 1) * 64, :],
                    in0=u[0:64, :], in1=rb[64:128, :])

            def emit_pair(cur, nxt, pT_cur, pT_next):
                """Interleave next pair's scores with current pair's AV
                chains at kt granularity, plus one filler step per
                iteration."""
                nkt_cur = 4 * cur[0] + 4 if cur else 0
                nkt_nxt = 4 * nxt[0] + 4 if nxt else 0
                u = {}
                ao = None
                if cur:
                    ao = aoT_pool.tile([128, 512], F16, tag="aoT", name="aoT")
                    for hh in (0, 1):
                        u[hh] = upsum.tile([128, 512], F32, tag="u", name="u")
                if cur and not nxt:
                    # Last pair: no next-pair scores to interleave.  Run the
                    # AV chains head-sequential and normalize each head as
                    # soon as its chain stops, so the normalize of head 0
                    # overlaps the PE chain of head 1.
                    for hh in (0, 1):
                        for kt in range(nkt_cur):
                            av_mm(u[hh], cur, kt, hh, nkt_cur, pT_cur)
                            emit_filler_step()
                        finalize_head(u[hh], ao, hh)
                    return ao
                for kt in range(max(nkt_cur, nkt_nxt)):
                    if kt < nkt_nxt:
                        emit_score_kt(nxt[0], nxt[1], kt, pT_next)
                    if kt < nkt_cur:
                        for hh in (0, 1):
                            av_mm(u[hh], cur, kt, hh, nkt_cur, pT_cur)
                    emit_filler_step()
                if cur:
                    for hh in (0, 1):
                        finalize_head(u[hh], ao, hh)
                return ao

            # ---- schedule ----------------------------------------------------
            # Prelude: block-0 projections and V chains in bulk (psum through
            # upsum so back-to-back chains double-buffer), dq-interleaved so
            # pair (0,0) unblocks after the first chains.
            for dq in range(HD // 128):
                emit_chain_bulk(0, dq, xs0, upsum)
            for tb in range(4):
                emit_v_bulk(tb, xs0, upsum)

            pairs = [(qb, hp) for qb in range(n_qb) for hp in range(HH // 2)]
            # out-projection slices are placed into the exp-heavy late pairs
            oproj_place = defaultdict(list)
            for qb in range(n_qb - 1):
                for qt_l in range(4):
                    # spill 2 units per pair at the end instead of lumping
                    idx = min(4 * qb + 6 + qt_l,
                              len(pairs) - 3 + (qt_l >= 2))
                    oproj_place[idx].append((qb, qt_l))

            pT_next = {}
            emit_pair(None, pairs[0], None, pT_next)
            ao_by_qb = {qb: [] for qb in range(n_qb)}
            xs_next = None
            for i, (qb, hp) in enumerate(pairs):
                pT_cur, pT_next = pT_next, {}
                nxt = pairs[i + 1] if i + 1 < len(pairs) else None
                if qb + 1 < n_qb and hp == 0:
                    xs_next = load_x_tiles(qb + 1)
                # queue filler: projection chains for the next block (one dq
                # per pair), V chains late in the block, out-projections per
                # the placement table.
                for oqb, qt_l in oproj_place.get(i, ()):
                    filler.extend(oproj_unit(oqb, ao_by_qb[oqb], qt_l, opsum))
                if qb + 1 < n_qb:
                    filler.extend(chain_steps(qb + 1, hp, xs_next))
                    if hp == HH // 2 - 1:
                        for tb in range(4 * (qb + 1), 4 * (qb + 1) + 4):
                            filler.extend(v_steps(tb, xs_next))
                ao_by_qb[qb].append(emit_pair((qb, hp), nxt, pT_cur, pT_next))
            while filler:
                emit_filler_step()
            # tail: out-projection of the last block (psum through upsum so
            # the four slices pipeline)
            for qt_l in range(4):
                for step in oproj_unit(n_qb - 1, ao_by_qb[n_qb - 1], qt_l, upsum):
                    step()

    nc.compile()
    return nc


_NC_CACHE = {}


def _get_nc(s=S):
    if s not in _NC_CACHE:
        _NC_CACHE[s] = build_nc(s)
    return _NC_CACHE[s]


def make_in_maps(x, w_q, w_k, w_v, w_o, s=S):
    """Host-side sharding: returns the 8 per-core input maps."""
    x = np.ascontiguousarray(np.asarray(x, dtype=np.float32))
    w_q = np.asarray(w_q, dtype=np.float32)
    w_k = np.asarray(w_k, dtype=np.float32)
    w_v = np.asarray(w_v, dtype=np.float32)
    w_o = np.asarray(w_o, dtype=np.float32)

    xTs = [np.ascontiguousarray(x[b].T.astype(np.float16)) for b in range(B)]
    wqTs = [np.ascontiguousarray(w_q[hg * HD:(hg + 1) * HD, :].T.astype(np.float16)) for hg in range(2)]
    wkTs = [np.ascontiguousarray(w_k[hg * HD:(hg + 1) * HD, :].T.astype(np.float16)) for hg in range(2)]
    wvTs = [np.ascontiguousarray(w_v[hg * HD:(hg + 1) * HD, :].T.astype(np.float16)) for hg in range(2)]
    woTs = [np.ascontiguousarray(w_o[:, hg * HD:(hg + 1) * HD].T.astype(np.float16)) for hg in range(2)]

    in_maps = []
    for c in range(N_CORES):
        b, hg = c // 2, c % 2
        in_maps.append({
            "xT": xTs[b], "wqT": wqTs[hg], "wkT": wkTs[hg],
            "wvT": wvTs[hg], "woT": woTs[hg],
        })
    return in_maps


def kernel(x, w_q, w_k, w_v, w_o, b_o):
    nc = _get_nc(S)
    in_maps = make_in_maps(x, w_q, w_k, w_v, w_o, s=S)
    res = run_bass_kernel_spmd(nc, in_maps, core_ids=list(range(N_CORES)))
    b_o = np.asarray(b_o, dtype=np.float32)
    outp = np.empty((B, S, D), dtype=np.float32)
    for b in range(B):
        outp[b] = (res.results[2 * b]["out"].astype(np.float32)
                   + res.results[2 * b + 1]["out"].astype(np.float32) + b_o)
    return outp
